# revision 1
# baseline (speedup 1.0000x reference)
"""Trainium2 Bass kernel for a 2-layer GAT (CORA-style) over 8 NeuronCores.

Architecture (1D node partition, edges assigned by destination core):
  - Node phase: per core, h1' = x_shard @ [W1 | w_asrc | w_adst] on the PE
    (host supplies x transposed). Rows land in a per-core Z1 shard
    ([h(64) | a_src(8) | a_dst(8)] per node, 512B row stride).
  - AllGather Z1 shards -> Z1_full replicated on every core.
  - Edge phase, layer 1: destination-sorted edge list, chunks of 128 edges
    on the SBUF partition dim. Per-edge source rows ([h|a_src], 288B payload)
    come from `dma_gather` (the MoE Q7 row-gather; int16 indices, so Z1_full
    is windowed into 4 quarter-ranges of 2 shards each). Per-edge a_dst
    (32B payload) is a second dma_gather from the core-local Z1 shard.
    Attention: exp(leaky_relu(a_src+a_dst)) via DVE/ACT (leaky = 0.6x+0.4|x|),
    then the segment softmax-sum and weighted feature sum are ONE one-hot
    matmul per chunk: psum[128 nodes, 72] += S[e,m]^T @ [exp | exp*h], where
    S = is_equal(iota, dst_slot) is built on the DVE. Tile postprocess:
    out1 = num/s + b, ELU, and the fused layer-2 projection (PE transpose +
    matmul) writing the Z2 shard ([g2(16)|a_src2|a_dst2], 256B stride).
  - AllGather Z2, then the same edge-phase structure for layer 2 (1 head),
    followed by the row softmax; both [N,16] outputs are written per core.

The host only builds index tables (int16 gather indices, slot ids) and
slices/concats the outputs; all FLOPs and data movement run on-device.
"""

import math
import os
from contextlib import ExitStack

import numpy as np

import os
import concourse.bass as bass
import concourse.bacc as bacc
import concourse.mybir as mybir
import concourse.tile as tile
from concourse import ap_utils
from concourse.bass import round_up_to_multiple, exact_div

NCORES = 8
P = 128
NQ = 4            # gather windows (2 shards each) -> int16-safe row indices
PADV = -150.0     # pad-row a_src: exp(leaky(-150 + 0)) ~ 8e-14

F32 = mybir.dt.float32
I16 = mybir.dt.int16
ALU = mybir.AluOpType
ACTF = mybir.ActivationFunctionType
AX = mybir.AxisListType

TG1 = 4    # dst tiles per layer-1 supergroup
TG2 = 4    # dst tiles per layer-2 supergroup
SB = 16    # chunks per one-hot build batch


def dma_gather_rows(g, out_ap, in_ap, idxs_ap, num_idxs, elem_size, elem_step,
                    queue_num=0):
    """bass.dma_gather minus the 256B-payload assert (the Q7 kernel packetizes
    arbitrary payloads; only the row stride must be a 256B multiple)."""
    assert idxs_ap.dtype == mybir.dt.int16
    assert in_ap.dtype == out_ap.dtype
    assert in_ap.space == bass.MemorySpace.DRAM
    assert idxs_ap.space == bass.MemorySpace.SBUF
    assert out_ap.space == bass.MemorySpace.SBUF
    assert ap_utils.ap_is_contiguous(out_ap.ap[1:])
    assert ap_utils.ap_is_contiguous(idxs_ap.ap[1:])
    assert in_ap.ap[-1][1] == out_ap.ap[-1][1] == elem_size
    assert out_ap.ap[0][1] * out_ap.ap[1][1] == round_up_to_multiple(num_idxs, 128)
    assert in_ap.ap[0][0] == elem_step
    stride_bytes_256 = exact_div(elem_step * mybir.dt.size(in_ap.dtype), 256)
    assert stride_bytes_256 < 256
    _in_ap = g.lower_ap_dma(in_ap, for_custom_bir_dma=True)
    _idxs_ap = g.lower_ap(idxs_ap)
    _out_ap = g.lower_ap(out_ap)
    return g.add_instruction(
        mybir.InstDMAGatherAnt(
            name=g.bass.get_next_instruction_name(),
            ins=[*_in_ap, _idxs_ap, g.lower_val_access(g.to_reg(num_idxs))],
            outs=[_out_ap],
            transpose=False,
            num_idxs=num_idxs,
            elem_size=elem_size,
            stride_bytes_256=stride_bytes_256,
            gen_mode=0,
            single_packet=bool(int(os.environ.get('GAT_SP','0'))),
            queue_num=queue_num,
            sbuf_tokens_per_rank=0,
            sbuf_free_dim_per_rank=0,
            sbuf_free_dim_pad_per_rank=0,
            sbuf_byte_offset=0,
        )
    )


def _wrap_idx(flat):
    """int16 idx list -> [128, n/16] SBUF layout (idx k at [k%16, k//16],
    replicated 8x across 16-partition groups)."""
    n = flat.size
    assert n % 16 == 0
    w = flat.reshape(-1, 16).T.astype(np.int16)      # [16, n/16]
    return np.tile(w, (8, 1))                        # [128, n/16]


class _EdgeSchedule:
    """Shared (SPMD) chunk schedule + per-core index tables for one layer."""

    def __init__(self, NT, wrows):
        self.NT = NT
        self.wrows = wrows          # rows per gather window
        self.CH = None              # [NT, NQ] chunks per (tile, window)
        self.sgs = []               # list of supergroups; see build_schedule

    def build_schedule(self, counts_by_core, TG):
        # counts_by_core: list of [NT, NQ] real-edge counts
        mx = np.max(np.stack(counts_by_core), axis=0)
        self.CH = np.ceil(mx / P).astype(np.int64)   # chunks per (t, q)
        sgs = []
        for t0 in range(0, self.NT, TG):
            tiles = list(range(t0, min(t0 + TG, self.NT)))
            qch = [(q, t, int(self.CH[t, q]))
                   for q in range(NQ) for t in tiles if self.CH[t, q] > 0]
            nch = sum(c for _, _, c in qch)
            sgs.append(dict(tiles=tiles, qch=qch, nch=nch))
        self.sgs = sgs

    def chunk_layout(self):
        """Global chunk order: per supergroup, per (q,t) run. Returns per-chunk
        (sg, q, t) and per-sg window chunk counts."""
        order = []
        for si, sg in enumerate(self.sgs):
            for q, t, c in sg["qch"]:
                for _ in range(c):
                    order.append((si, q, t))
        return order


def _prep(x, edge_index, W1, att_src1, att_dst1, bias1, W2, att_src2, att_dst2,
          bias2):
    N, F = x.shape
    HID = W1.shape[1]
    H1, O1 = att_src1.shape
    C = W2.shape[1]
    assert N % NCORES == 0
    NSH = N // NCORES
    NT = math.ceil(NSH / P)
    NPAD = NT * P
    NR1 = NSH + 1
    NR2 = NPAD + 1
    W1ROW = 128                      # Z1 row stride (f32) = 512B
    W2ROW = 64                       # Z2 row stride (f32) = 256B
    WIN1 = 2 * NR1                   # rows per z1 gather window
    WIN2 = 2 * NR2
    assert WIN1 <= 32767 and WIN2 <= 32767

    x = np.asarray(x, np.float32)
    W1 = np.asarray(W1, np.float32)
    W2 = np.asarray(W2, np.float32)
    att_src1 = np.asarray(att_src1, np.float32)
    att_dst1 = np.asarray(att_dst1, np.float32)
    att_src2 = np.asarray(att_src2, np.float32)
    att_dst2 = np.asarray(att_dst2, np.float32)
    bias1 = np.asarray(bias1, np.float32).reshape(-1)
    bias2 = np.asarray(bias2, np.float32).reshape(-1)

    W1r = W1.reshape(F, H1, O1)
    w_asrc1 = np.einsum("fhc,hc->fh", W1r, att_src1)
    w_adst1 = np.einsum("fhc,hc->fh", W1r, att_dst1)
    W1p = np.concatenate([W1, w_asrc1, w_adst1], axis=1).astype(np.float32)
    w_asrc2 = W2 @ att_src2[0]
    w_adst2 = W2 @ att_dst2[0]
    W2p = np.concatenate([W2, w_asrc2[:, None], w_adst2[:, None]], axis=1).astype(
        np.float32
    )
    b1rep = np.broadcast_to(bias1, (P, HID)).copy()
    b2rep = np.broadcast_to(bias2, (P, C)).copy()
    iota = np.broadcast_to(np.arange(P, dtype=np.float32), (P, P)).copy()

    src = np.asarray(edge_index[0], np.int64)
    dst = np.asarray(edge_index[1], np.int64)
    loop = np.arange(N, dtype=np.int64)
    src = np.concatenate([src, loop])
    dst = np.concatenate([dst, loop])
    core_of = dst // NSH

    z1row = src + src // NSH                                   # row in Z1_full
    z2row = (src // NSH) * NR2 + (src % NSH)                   # row in Z2_full
    q1 = z1row // WIN1
    q2 = z2row // WIN2

    sched1 = _EdgeSchedule(NT, WIN1)
    sched2 = _EdgeSchedule(NT, WIN2)

    per_core = []
    for c in range(NCORES):
        m = core_of == c
        d_loc = dst[m] - c * NSH
        t_of = d_loc // P
        e = dict(
            d_loc=d_loc, t_of=t_of,
            z1=(z1row[m] - q1[m] * WIN1), q1=q1[m],
            z2=(z2row[m] - q2[m] * WIN2), q2=q2[m],
        )
        cnt1 = np.zeros((NT, NQ), np.int64)
        cnt2 = np.zeros((NT, NQ), np.int64)
        np.add.at(cnt1, (t_of, e["q1"]), 1)
        np.add.at(cnt2, (t_of, e["q2"]), 1)
        e["cnt1"], e["cnt2"] = cnt1, cnt2
        per_core.append(e)

    sched1.build_schedule([pc["cnt1"] for pc in per_core], TG1)
    sched2.build_schedule([pc["cnt2"] for pc in per_core], TG2)

    def build_tables(sched, key_q, key_z, pad_src, pad_dst, NR):
        """Per-core gather index + slot tables following the shared schedule."""
        gidx_parts, dslot_parts, didx_parts = [], [], []
        for pc in per_core:
            t_of, d_loc = pc["t_of"], pc["d_loc"]
            qv, zv = pc[key_q], pc[key_z]
            # bucket edges by (tile, window)
            order = np.lexsort((qv, t_of))
            ts, qs, zs, ds = t_of[order], qv[order], zv[order], d_loc[order]
            # start offset of each (t, q) run
            keys = ts * NQ + qs
            starts = np.searchsorted(keys, np.arange(NT * NQ))
            ends = np.searchsorted(keys, np.arange(NT * NQ) + 1)
            g_sg, s_sg, d_sg = [], [], []
            for sg in sched.sgs:
                g_q = {q: [] for q in range(NQ)}
                s_all, d_all = {q: [] for q in range(NQ)}, {q: [] for q in range(NQ)}
                for q, t, ch in sg["qch"]:
                    k = t * NQ + q
                    a, b = starts[k], ends[k]
                    nslots = ch * P
                    gi = np.full(nslots, pad_src, np.int64)
                    sl = np.full(nslots, -1.0, np.float32)
                    di = np.full(nslots, NR - 1, np.int64)
                    gi[: b - a] = zs[a:b]
                    sl[: b - a] = (ds[a:b] % P).astype(np.float32)
                    di[: b - a] = ds[a:b]
                    g_q[q].append(gi)
                    s_all[q].append(sl)
                    d_all[q].append(di)
                sg_g = [np.concatenate(g_q[q]) if g_q[q] else
                        np.zeros(0, np.int64) for q in range(NQ)]
                sg_s = np.concatenate([s for q in range(NQ) for s in s_all[q]]) \
                    if sg["nch"] else np.zeros(0, np.float32)
                sg_d = np.concatenate([d for q in range(NQ) for d in d_all[q]]) \
                    if sg["nch"] else np.zeros(0, np.int64)
                g_sg.append(sg_g)
                s_sg.append(sg_s)
                d_sg.append(sg_d)
            # flatten to DRAM arrays: per sg: [wrap(q0)|wrap(q1)|..|wrap(dst)]
            gw, sw = [], []
            for si, sg in enumerate(sched.sgs):
                for q in range(NQ):
                    if g_sg[si][q].size:
                        gw.append(_wrap_idx(g_sg[si][q]))
                if sg["nch"]:
                    gw.append(_wrap_idx(d_sg[si]))
                    sw.append(s_sg[si].reshape(-1, P).T)   # [128, nch]
            gidx_parts.append(np.concatenate(gw, axis=1) if gw else
                              np.zeros((P, 0), np.int16))
            dslot_parts.append(np.concatenate(sw, axis=1) if sw else
                               np.zeros((P, 0), np.float32))
            didx_parts.append(None)
        return gidx_parts, dslot_parts

    gidx1, dslot1 = build_tables(sched1, "q1", "z1", NSH, None, NR1)
    gidx2, dslot2 = build_tables(sched2, "q2", "z2", NPAD, None, NR2)

    xT = np.ascontiguousarray(x.T)
    in_maps = []
    for c in range(NCORES):
        xsh = np.zeros((F, NPAD), np.float32)
        xsh[:, :NSH] = xT[:, c * NSH : (c + 1) * NSH]
        in_maps.append(
            {
                "xT": xsh, "w1p": W1p, "w2p": W2p, "b1": b1rep, "b2": b2rep,
                "iota": iota,
                "ident": np.eye(P, dtype=np.float32),
                "gidx1": gidx1[c].astype(np.int16),
                "dslot1": dslot1[c].astype(np.float32),
                "gidx2": gidx2[c].astype(np.int16),
                "dslot2": dslot2[c].astype(np.float32),
            }
        )
    shapes = {k: v.shape for k, v in in_maps[0].items()}
    for im in in_maps:
        for k in im:
            assert im[k].shape == shapes[k], (k, im[k].shape, shapes[k])

    meta = dict(
        N=N, F=F, HID=HID, H1=H1, O1=O1, C=C, NSH=NSH, NT=NT, NPAD=NPAD,
        NR1=NR1, NR2=NR2, W1ROW=W1ROW, W2ROW=W2ROW, WIN1=WIN1, WIN2=WIN2,
        sched1=sched1, sched2=sched2,
        G1=gidx1[0].shape[1], S1=dslot1[0].shape[1],
        G2=gidx2[0].shape[1], S2=dslot2[0].shape[1],
    )
    return meta, in_maps


def _build(meta):
    F, HID, H1, O1, C = meta["F"], meta["HID"], meta["H1"], meta["O1"], meta["C"]
    NSH, NT, NPAD = meta["NSH"], meta["NT"], meta["NPAD"]
    NR1, NR2 = meta["NR1"], meta["NR2"]
    W1ROW, W2ROW = meta["W1ROW"], meta["W2ROW"]
    WIN1, WIN2 = meta["WIN1"], meta["WIN2"]
    sched1, sched2 = meta["sched1"], meta["sched2"]
    KCH = F // P
    ZC1 = HID + 2 * H1          # 80 used cols in z1 rows
    PAY1 = HID + H1             # 72: [h | asrc] src payload
    ZC2 = C + 2                 # 18 used cols in z2 rows
    PAY2 = C + 1                # 17: [g2 | asrc2]

    nc = bacc.Bacc(
        "TRN2", target_bir_lowering=False, debug=False,
        enable_asserts=False, num_devices=NCORES,
    )

    xT = nc.dram_tensor("xT", [F, NPAD], F32, kind="ExternalInput")
    w1p = nc.dram_tensor("w1p", [F, ZC1], F32, kind="ExternalInput")
    w2p = nc.dram_tensor("w2p", [HID, ZC2], F32, kind="ExternalInput")
    b1 = nc.dram_tensor("b1", [P, HID], F32, kind="ExternalInput")
    b2 = nc.dram_tensor("b2", [P, C], F32, kind="ExternalInput")
    iot = nc.dram_tensor("iota", [P, P], F32, kind="ExternalInput")
    idn = nc.dram_tensor("ident", [P, P], F32, kind="ExternalInput")
    gidx1 = nc.dram_tensor("gidx1", [P, meta["G1"]], I16, kind="ExternalInput")
    dslot1 = nc.dram_tensor("dslot1", [P, meta["S1"]], F32, kind="ExternalInput")
    gidx2 = nc.dram_tensor("gidx2", [P, meta["G2"]], I16, kind="ExternalInput")
    dslot2 = nc.dram_tensor("dslot2", [P, meta["S2"]], F32, kind="ExternalInput")
    out_sm = nc.dram_tensor("out_sm", [NPAD, C], F32, kind="ExternalOutput")
    out_emb = nc.dram_tensor("out_emb", [NPAD, C], F32, kind="ExternalOutput")

    z1sh = nc.dram_tensor("z1sh", [NR1, W1ROW], F32)
    z1full = nc.dram_tensor("z1full", [NCORES * NR1, W1ROW], F32, addr_space="Shared")
    z2sh = nc.dram_tensor("z2sh", [NR2, W2ROW], F32)
    z2full = nc.dram_tensor("z2full", [NCORES * NR2, W2ROW], F32, addr_space="Shared")
    rg = [list(range(NCORES))]

    with tile.TileContext(nc, num_cores=NCORES) as tc, ExitStack() as ctx:
        cpool = ctx.enter_context(tc.tile_pool(name="const", bufs=1))
        w1sb = cpool.tile([P, KCH, ZC1], F32)
        nc.sync.dma_start(w1sb[:], w1p[:].rearrange("(k p) o -> p k o", p=P))
        w2sb = cpool.tile([HID, ZC2], F32)
        nc.sync.dma_start(w2sb[:], w2p[:])
        b1sb = cpool.tile([P, HID], F32)
        nc.sync.dma_start(b1sb[:], b1[:])
        b2sb = cpool.tile([P, C], F32)
        nc.sync.dma_start(b2sb[:], b2[:])
        iosb = cpool.tile([P, P], F32)
        nc.sync.dma_start(iosb[:], iot[:])
        ident = cpool.tile([P, P], F32)
        nc.sync.dma_start(ident[:], idn[:])

        # pad rows
        pr1 = cpool.tile([1, W1ROW], F32)
        nc.vector.memset(pr1[:], 0.0)
        nc.vector.memset(pr1[:, HID : HID + H1], PADV)
        nc.sync.dma_start(z1sh[NSH : NSH + 1, :], pr1[:])
        pr2 = cpool.tile([1, W2ROW], F32)
        nc.vector.memset(pr2[:], 0.0)
        nc.vector.memset(pr2[:, C : C + 1], PADV)
        nc.sync.dma_start(z2sh[NPAD : NPAD + 1, :], pr2[:])

        # ---------------- phase A: layer-1 node projection ----------------
        with (
            tc.tile_pool(name="nodeA", bufs=3) as pa,
            tc.tile_pool(name="psumA", bufs=2, space="PSUM") as ppa,
        ):
            for t in range(NT):
                xt = pa.tile([P, KCH, P], F32, tag="xt")
                nc.sync.dma_start(
                    xt[:], xT[:, t * P : (t + 1) * P].rearrange("(k p) n -> p k n", p=P)
                )
                ps = ppa.tile([P, ZC1], F32, tag="h1ps")
                for k in range(KCH):
                    nc.tensor.matmul(
                        ps[:], lhsT=xt[:, k, :], rhs=w1sb[:, k, :],
                        start=(k == 0), stop=(k == KCH - 1),
                    )
                zt = pa.tile([P, W1ROW], F32, tag="zt")
                nc.scalar.copy(zt[:, 0:ZC1], ps[:])
                nc.vector.memset(zt[:, ZC1:], 0.0)
                rows = min(NSH - t * P, P)
                nc.sync.dma_start(z1sh[t * P : t * P + rows, :], zt[:rows, :])

        if int(os.environ.get("GAT_PHASES", "5")) >= 2:
            nc.gpsimd.collective_compute(
                "AllGather", ALU.bypass, replica_groups=rg,
                ins=[z1sh[:]], outs=[z1full[:]]
            )

        # ---------------- layer-1 edge phase (+ fused layer-2 projection) ----
        def edge_phase(sched, zfull, zloc, WIN, PAY, ZROW, gidx, dslot, goff, soff,
                       NH, OC, layer):
            """NH heads x OC channels; layer 1 or 2 (different postprocess)."""
            VAL = NH + NH * OC       # [exp(NH) | exp*h(NH*OC)]
            with (
                tc.tile_pool(name=f"eg{layer}", bufs=2) as pg,
                tc.tile_pool(name=f"es{layer}", bufs=2) as ps_,
                tc.tile_pool(name=f"ep{layer}", bufs=4, space="PSUM") as pp,
                tc.tile_pool(name=f"ep2{layer}", bufs=2, space="PSUM") as pp2,
            ):
                for sg in sched.sgs:
                    nch = sg["nch"]
                    if nch == 0:
                        continue
                    # idx slab for this sg: NQ windows then dst gather
                    widths = [c for _, _, c in sg["qch"]]
                    gg = pg.tile([P, nch, PAY], F32, tag="gg")
                    dg = ps_.tile([P, nch, NH], F32, tag="dg")
                    sl = ps_.tile([P, nch], F32, tag="sl")
                    nc.sync.dma_start(sl[:], dslot[:, soff[0] : soff[0] + nch])
                    soff[0] += nch
                    # per-window source gathers
                    qruns = {}
                    for q, t, c in sg["qch"]:
                        qruns.setdefault(q, 0)
                        qruns[q] += c
                    off = 0
                    for q in range(NQ):
                        cq = qruns.get(q, 0)
                        if cq == 0:
                            continue
                        n = cq * P
                        ixw = ps_.tile([P, n // 16], I16, tag="ixw")
                        nc.sync.dma_start(
                            ixw[:], gidx[:, goff[0] : goff[0] + n // 16]
                        )
                        goff[0] += n // 16
                        if int(os.environ.get("GAT_SRCG", "1")): dma_gather_rows(
                            nc.gpsimd,
                            out_ap=gg[:, off : off + cq, :],
                            in_ap=zfull[q * WIN : (q + 1) * WIN, 0:PAY],
                            idxs_ap=ixw[:],
                            num_idxs=n,
                            elem_size=PAY,
                            elem_step=ZROW,
                            queue_num=0,
                        )
                        off += cq
                    # dst gather (a_dst cols of the local shard)
                    n = nch * P
                    ixd = ps_.tile([P, n // 16], I16, tag="ixd")
                    nc.sync.dma_start(ixd[:], gidx[:, goff[0] : goff[0] + n // 16])
                    goff[0] += n // 16
                    if int(os.environ.get("GAT_DSTG", "1")): dma_gather_rows(
                        nc.gpsimd,
                        out_ap=dg[:],
                        in_ap=(zloc[:, 0:NH] if int(os.environ.get("GAT_DOFF0", "0"))
                               else zloc[:, PAY : PAY + NH]),
                        idxs_ap=ixd[:],
                        num_idxs=n,
                        elem_size=NH,
                        elem_step=ZROW,
                        queue_num=0,
                    )
                    EP = int(os.environ.get("GAT_EDGE", "5"))
                    if EP < 2:
                        continue
                    # attention: z = asrc + adst ; alpha = 0.6 z + 0.4|z| ; exp
                    zt = ps_.tile([P, nch, NH], F32, tag="zt")
                    nc.vector.tensor_tensor(
                        out=zt[:], in0=gg[:, :, NH * OC : NH * OC + NH], in1=dg[:],
                        op=ALU.add,
                    )
                    ab = ps_.tile([P, nch, NH], F32, tag="ab")
                    nc.scalar.activation(ab[:], zt[:], ACTF.Abs, scale=0.4)
                    al = ps_.tile([P, nch, NH], F32, tag="al")
                    nc.vector.scalar_tensor_tensor(
                        out=al[:], in0=zt[:], scalar=0.6, in1=ab[:],
                        op0=ALU.mult, op1=ALU.add,
                    )
                    val = pg.tile([P, nch, VAL], F32, tag="val")
                    nc.scalar.activation(val[:, :, 0:NH], al[:], ACTF.Exp)
                    nc.vector.tensor_tensor(
                        out=val[:, :, NH:].rearrange("p n (h c) -> p n h c", h=NH),
                        in0=gg[:, :, 0 : NH * OC].rearrange(
                            "p n (h c) -> p n h c", h=NH
                        ),
                        in1=val[:, :, 0:NH][:, :, :, None].to_broadcast(
                            [P, nch, NH, OC]
                        ),
                        op=ALU.mult,
                    )
                    if EP < 3:
                        continue
                    # one-hot scatter matmuls, batched S build
                    tiles_ps = {}
                    chunk_meta = []   # (chunk pos, tile)
                    for q, t, c in sg["qch"]:
                        for _ in range(c):
                            chunk_meta.append(t)
                    remaining = {t: chunk_meta.count(t) for t in set(chunk_meta)}
                    started = set()
                    for c0 in range(0, nch, SB):
                        cb = min(SB, nch - c0)
                        st = ps_.tile([P, SB, P], F32, tag="st")
                        nc.vector.tensor_tensor(
                            out=st[:, :cb, :],
                            in0=iosb[:][:, None, :].to_broadcast([P, cb, P]),
                            in1=sl[:, c0 : c0 + cb][:, :, None].to_broadcast(
                                [P, cb, P]
                            ),
                            op=ALU.is_equal,
                        )
                        for j in range(cb):
                            t = chunk_meta[c0 + j]
                            if t not in tiles_ps:
                                tiles_ps[t] = pp.tile(
                                    [P, VAL], F32, tag="acc", name=f"acc_t{t}"
                                )
                            remaining[t] -= 1
                            nc.tensor.matmul(
                                tiles_ps[t][:],
                                lhsT=st[:, j, :],
                                rhs=val[:, c0 + j, :],
                                start=(t not in started),
                                stop=(remaining[t] == 0),
                            )
                            started.add(t)
                    if EP < 4:
                        continue
                    # tile postprocess
                    for t in sg["tiles"]:
                        if t not in tiles_ps:
                            continue
                        acc = tiles_ps[t]
                        s2 = ps_.tile([P, NH], F32, tag="s2")
                        nc.vector.tensor_scalar_add(s2[:], acc[:, 0:NH], 1e-16)
                        rc = ps_.tile([P, NH], F32, tag="rc")
                        nc.vector.reciprocal(rc[:], s2[:])
                        o1 = ps_.tile([P, NH, OC], F32, tag="o1")
                        nc.vector.tensor_tensor(
                            out=o1[:],
                            in0=acc[:, NH:].rearrange("p (h c) -> p h c", h=NH),
                            in1=rc[:][:, :, None].to_broadcast([P, NH, OC]),
                            op=ALU.mult,
                        )
                        if layer == 1:
                            o2 = ps_.tile([P, HID], F32, tag="o2")
                            nc.vector.tensor_tensor(
                                out=o2[:], in0=o1[:].rearrange("p h c -> p (h c)"),
                                in1=b1sb[:], op=ALU.add,
                            )
                            mn = ps_.tile([P, HID], F32, tag="mn")
                            nc.vector.tensor_scalar_min(mn[:], o2[:], 0.0)
                            exm = ps_.tile([P, HID], F32, tag="exm")
                            nc.scalar.activation(exm[:], mn[:], ACTF.Exp)
                            h2a = ps_.tile([P, HID], F32, tag="h2a")
                            nc.vector.scalar_tensor_tensor(
                                out=h2a[:], in0=o2[:], scalar=0.0, in1=exm[:],
                                op0=ALU.max, op1=ALU.add,
                            )
                            h2t = ps_.tile([P, HID], F32, tag="h2t")
                            nc.vector.tensor_scalar_add(h2t[:], h2a[:], -1.0)
                            if EP < 5:
                                continue
                            tp = pp2.tile([HID, P], F32, tag="h2T")
                            nc.tensor.transpose(tp[:], h2t[:], ident[:])
                            h2T = ps_.tile([HID, P], F32, tag="h2Ts")
                            nc.scalar.copy(h2T[:], tp[:])
                            g2p = pp2.tile([P, ZC2], F32, tag="g2ps")
                            nc.tensor.matmul(
                                g2p[:], lhsT=h2T[:], rhs=w2sb[:], start=True,
                                stop=True,
                            )
                            z2t = ps_.tile([P, W2ROW], F32, tag="z2t")
                            nc.scalar.copy(z2t[:, 0:ZC2], g2p[:])
                            nc.vector.memset(z2t[:, ZC2:], 0.0)
                            nc.sync.dma_start(z2sh[t * P : (t + 1) * P, :], z2t[:])
                        else:
                            emb = ps_.tile([P, C], F32, tag="emb")
                            nc.vector.tensor_tensor(
                                out=emb[:], in0=o1[:].rearrange("p h c -> p (h c)"),
                                in1=b2sb[:], op=ALU.add,
                            )
                            mxn = ps_.tile([P, 1], F32, tag="mxn")
                            nc.vector.tensor_reduce(
                                mxn[:], emb[:], axis=AX.X, op=ALU.max, negate=True
                            )
                            es = ps_.tile([P, C], F32, tag="es")
                            ssum = ps_.tile([P, 1], F32, tag="ssq")
                            nc.scalar.activation(
                                es[:], emb[:], ACTF.Exp, bias=mxn[:, :1],
                                accum_out=ssum[:],
                            )
                            rr = ps_.tile([P, 1], F32, tag="rr")
                            nc.vector.reciprocal(rr[:], ssum[:])
                            sm = ps_.tile([P, C], F32, tag="sm")
                            nc.vector.tensor_scalar(
                                sm[:], es[:], rr[:, :1], None, op0=ALU.mult
                            )
                            nc.sync.dma_start(out_emb[t * P : (t + 1) * P, :], emb[:])
                            nc.sync.dma_start(out_sm[t * P : (t + 1) * P, :], sm[:])

        PH = int(os.environ.get("GAT_PHASES", "5"))
        if PH >= 3:
            goff1, soff1 = [0], [0]
            edge_phase(sched1, z1full, z1sh, WIN1, PAY1, W1ROW, gidx1, dslot1,
                       goff1, soff1, H1, O1, layer=1)
            assert goff1[0] == meta["G1"] and soff1[0] == meta["S1"]

        if PH >= 4:
            nc.gpsimd.collective_compute(
                "AllGather", ALU.bypass, replica_groups=rg,
                ins=[z2sh[:]], outs=[z2full[:]]
            )

        if PH >= 5:
            goff2, soff2 = [0], [0]
            edge_phase(sched2, z2full, z2sh, WIN2, PAY2, W2ROW, gidx2, dslot2,
                       goff2, soff2, 1, C, layer=2)
            assert goff2[0] == meta["G2"] and soff2[0] == meta["S2"]

    nc.compile()
    return nc


def _postprocess(meta, results):
    NSH = meta["NSH"]
    outs, embs = [], []
    for c in range(NCORES):
        outs.append(results[c]["out_sm"][:NSH])
        embs.append(results[c]["out_emb"][:NSH])
    return np.concatenate(outs, 0), np.concatenate(embs, 0)


def kernel(**inputs):
    meta, in_maps = _prep(**inputs)
    nc = _build(meta)
    from concourse.bass_utils import run_bass_kernel_spmd

    trace = bool(int(os.environ.get("GAT_TRACE", "0")))
    res = run_bass_kernel_spmd(nc, in_maps, list(range(NCORES)), trace=trace)
    if trace and res.exec_time_ns is not None:
        print(f"HW exec time: {res.exec_time_ns} ns")
        kernel.last_exec_time_ns = res.exec_time_ns
    return _postprocess(meta, res.results)



# revision 6
# speedup vs baseline: 2.1066x; 2.1066x over previous
"""Trainium2 Bass kernel for a 2-layer GAT (CORA-style) over 8 NeuronCores.

v2 — Q7-gather-minimized edge phase:
  - Node phase: per core, z1' = x_shard @ [W1 | w_asrc | w_adst] on the PE in
    bf16 (host supplies x transposed, bf16). Rows land in a per-core Z1 shard
    ([h(64) | a_src(8) | a_dst(8)] bf16, 256B row stride).
  - AllGather Z1 shards -> Z1_full replicated (bf16, half the bytes of v1).
  - Edge phase: destination-sorted edges, chunks of 128 on the partition dim.
    ONLY the per-edge source rows ([h|a_src], 144B) use the Q7 `dma_gather`
    (the serial ~9.5ns/row GpSimd descriptor bottleneck). Everything per-dst
    is derived locally:
      * a_dst per edge = one-hot expand on the PE: S^T[m,e] @ a_dst_tile[m].
        S^T is built WITHOUT PE transposes via a block-swapped is_equal
        (host-provided slh table) + DVE StreamTranspose (32x32 blocks).
      * self-loop edges (1 chunk per dst tile) skip the gather entirely:
        their source rows are the local shard tile, loaded by direct DMA.
    Attention exp(leaky()) on DVE/ACT in batches of SB=32 chunks; segment
    softmax-sum + weighted feature sum stay ONE one-hot matmul per chunk
    (bf16 lhsT/rhs, f32 psum).
  - Postprocess (softmax-normalize, bias, ELU, fused layer-2 projection /
    final row softmax) is batched per supergroup (TG=4 dst tiles) so DVE
    sees ~6 medium ops per sg instead of ~24 tiny (2-6us overhead) ops.
The host only builds index/slot tables and slices outputs.
"""

import math
import os
from contextlib import ExitStack

import numpy as np
import ml_dtypes

import concourse.bass as bass
import concourse.bacc as bacc
import concourse.mybir as mybir
import concourse.tile as tile
from concourse import ap_utils
from concourse.bass import round_up_to_multiple, exact_div

NCORES = 8
P = 128
NQ = 4            # gather windows (2 shards each) -> int16-safe row indices
PADV = -150.0     # pad-row a_src: exp(leaky(-150)) ~ 0

F32 = mybir.dt.float32
BF16 = mybir.dt.bfloat16
I16 = mybir.dt.int16
ALU = mybir.AluOpType
ACTF = mybir.ActivationFunctionType
AX = mybir.AxisListType
NPBF16 = ml_dtypes.bfloat16

TG = 4    # dst tiles per supergroup
SB = 32   # chunks per attention batch


def dma_gather_rows(g, out_ap, in_ap, idxs_ap, num_idxs, elem_size, elem_step,
                    queue_num=0):
    """bass.dma_gather minus the 256B-payload assert (the Q7 kernel packetizes
    arbitrary payloads; only the row stride must be a 256B multiple)."""
    assert idxs_ap.dtype == mybir.dt.int16
    assert in_ap.dtype == out_ap.dtype
    assert in_ap.space == bass.MemorySpace.DRAM
    assert idxs_ap.space == bass.MemorySpace.SBUF
    assert out_ap.space == bass.MemorySpace.SBUF
    assert ap_utils.ap_is_contiguous(out_ap.ap[1:])
    assert ap_utils.ap_is_contiguous(idxs_ap.ap[1:])
    assert in_ap.ap[-1][1] == out_ap.ap[-1][1] == elem_size
    assert out_ap.ap[0][1] * out_ap.ap[1][1] == round_up_to_multiple(num_idxs, 128)
    assert in_ap.ap[0][0] == elem_step
    stride_bytes_256 = exact_div(elem_step * mybir.dt.size(in_ap.dtype), 256)
    assert stride_bytes_256 < 256
    _in_ap = g.lower_ap_dma(in_ap, for_custom_bir_dma=True)
    _idxs_ap = g.lower_ap(idxs_ap)
    _out_ap = g.lower_ap(out_ap)
    return g.add_instruction(
        mybir.InstDMAGatherAnt(
            name=g.bass.get_next_instruction_name(),
            ins=[*_in_ap, _idxs_ap, g.lower_val_access(g.to_reg(num_idxs))],
            outs=[_out_ap],
            transpose=False,
            num_idxs=num_idxs,
            elem_size=elem_size,
            stride_bytes_256=stride_bytes_256,
            gen_mode=0,
            single_packet=False,
            queue_num=queue_num,
            sbuf_tokens_per_rank=0,
            sbuf_free_dim_per_rank=0,
            sbuf_free_dim_pad_per_rank=0,
            sbuf_byte_offset=0,
        )
    )


def _wrap_idx(flat):
    """int16 idx list -> [128, n/16] SBUF layout (idx k at [k%16, k//16],
    replicated 8x across 16-partition groups)."""
    n = flat.size
    assert n % 16 == 0
    w = flat.reshape(-1, 16).T.astype(np.int16)      # [16, n/16]
    return np.tile(w, (8, 1))                        # [128, n/16]


class _Layer:
    """Shared (SPMD) chunk schedule + per-core tables for one layer."""

    def __init__(self, NT, NSH):
        self.NT = NT
        self.NSH = NSH
        self.sgs = []     # [{tiles, rows, qruns, nch, chunk_tile, ngath}]
        self.G = 0        # gidx cols
        self.S = 0        # slot cols (chunks)

    def rows_of(self, t):
        return min(self.NSH - t * P, P)

    def build(self, counts_by_core):
        NT = self.NT
        CH = np.ceil(np.max(np.stack(counts_by_core), 0) / P).astype(np.int64)
        self.CH = CH
        for t0 in range(0, NT, TG):
            tiles = list(range(t0, min(t0 + TG, NT)))
            qch = [(q, t, int(CH[t, q]))
                   for q in range(NQ) for t in tiles if CH[t, q] > 0]
            qruns = {}
            for q, _, c in qch:
                qruns[q] = qruns.get(q, 0) + c
            ngath = sum(c for _, _, c in qch)
            nch = ngath + len(tiles)
            chunk_tile = []
            for q, t, c in qch:
                chunk_tile += [tiles.index(t)] * c
            chunk_tile += list(range(len(tiles)))      # self chunks
            self.sgs.append(dict(tiles=tiles, rows=[self.rows_of(t) for t in tiles],
                                 qch=qch, qruns=qruns, nch=nch, ngath=ngath,
                                 chunk_tile=chunk_tile))


def _build_layer_tables(layer, per_core_edges, pad_row):
    """Per-core gidx (int16 wrapped), dslot [P,S], dslh [P,4*S] tables."""
    NT, NQn = layer.NT, NQ
    gidx_parts, dsl_parts, slh_parts = [], [], []
    hsel = (np.arange(NQn)[None, :] * 32 + (np.arange(P) % 32)[:, None])  # [128,4]
    for ed in per_core_edges:
        t_of, d_loc, qv, zv = ed["t_of"], ed["d_loc"], ed["q"], ed["z"]
        order = np.lexsort((qv, t_of))
        ts, qs, zs, ds = t_of[order], qv[order], zv[order], d_loc[order]
        keys = ts * NQn + qs
        starts = np.searchsorted(keys, np.arange(NT * NQn))
        ends = np.searchsorted(keys, np.arange(NT * NQn) + 1)
        gw, slcols = [], []
        for sg in layer.sgs:
            for q, t, ch in sg["qch"]:
                k = t * NQn + q
                a, b = starts[k], ends[k]
                nslots = ch * P
                gi = np.full(nslots, pad_row, np.int64)
                sl = np.full(nslots, -1.0, np.float32)
                assert b - a <= nslots
                gi[: b - a] = zs[a:b]
                sl[: b - a] = (ds[a:b] % P).astype(np.float32)
                gw.append(_wrap_idx(gi))
                slcols.append(sl.reshape(-1, P).T)          # [128, ch]
            for i, t in enumerate(sg["tiles"]):             # self chunks
                rows = sg["rows"][i]
                sl = np.full(P, -1.0, np.float32)
                sl[:rows] = np.arange(rows, dtype=np.float32)
                slcols.append(sl[:, None])
        gidx = np.concatenate(gw, axis=1) if gw else np.zeros((P, 0), np.int16)
        slm = np.concatenate(slcols, axis=1)                # [128, S]
        # slh[p, j, h] = slm[h*32 + p%32, j]
        slh = slm[hsel, :]                                  # [128, 4, S]
        slh = np.ascontiguousarray(np.transpose(slh, (0, 2, 1))).reshape(P, -1)
        gidx_parts.append(gidx.astype(np.int16))
        dsl_parts.append(slm.astype(np.float32))
        slh_parts.append(slh.astype(np.float32))
    layer.G = gidx_parts[0].shape[1]
    layer.S = dsl_parts[0].shape[1]
    for g, s, h in zip(gidx_parts, dsl_parts, slh_parts):
        assert g.shape[1] == layer.G and s.shape[1] == layer.S
        assert h.shape[1] == 4 * layer.S
    return gidx_parts, dsl_parts, slh_parts


def _bf16(a):
    return np.asarray(a, dtype=NPBF16)


def _prep(x, edge_index, W1, att_src1, att_dst1, bias1, W2, att_src2, att_dst2,
          bias2):
    N, F = x.shape
    HID = W1.shape[1]
    H1, O1 = att_src1.shape
    C = W2.shape[1]
    assert N % NCORES == 0
    NSH = N // NCORES
    NT = math.ceil(NSH / P)
    NPAD = NT * P
    NR1 = NSH + 1
    NR2 = NPAD + 1
    ZROW = 128                       # z row stride in bf16 elements (256B)
    WIN1 = 2 * NR1
    WIN2 = 2 * NR2
    assert WIN1 <= 32767 and WIN2 <= 32767
    PAY1 = HID + H1                  # 72: [h | asrc]
    PAY2 = C + 1                     # 17: [g2 | asrc2]

    x = np.asarray(x, np.float32)
    W1 = np.asarray(W1, np.float32)
    W2 = np.asarray(W2, np.float32)
    att_src1 = np.asarray(att_src1, np.float32)
    att_dst1 = np.asarray(att_dst1, np.float32)
    att_src2 = np.asarray(att_src2, np.float32)
    att_dst2 = np.asarray(att_dst2, np.float32)
    bias1 = np.asarray(bias1, np.float32).reshape(-1)
    bias2 = np.asarray(bias2, np.float32).reshape(-1)

    W1r = W1.reshape(F, H1, O1)
    w_asrc1 = np.einsum("fhc,hc->fh", W1r, att_src1)
    w_adst1 = np.einsum("fhc,hc->fh", W1r, att_dst1)
    W1p = np.concatenate([W1, w_asrc1, w_adst1], axis=1)        # [F, 80]
    w_asrc2 = W2 @ att_src2[0]
    w_adst2 = W2 @ att_dst2[0]
    W2p = np.concatenate([W2, w_asrc2[:, None], w_adst2[:, None]], axis=1)
    b1rep = np.broadcast_to(bias1, (P, HID)).copy()
    b2rep = np.broadcast_to(bias2, (P, C)).copy()
    iota = np.broadcast_to(np.arange(P, dtype=np.float32), (P, P)).copy()
    # iota2[p, h*32+b] = (p//32)*32 + b  (block-swapped iota for the S^T build)
    iota2 = ((np.arange(P)[:, None] // 32) * 32
             + np.broadcast_to(np.arange(P)[None, :] % 32, (P, P))).astype(
        np.float32)
    identb = np.eye(P, dtype=np.float32)

    src = np.asarray(edge_index[0], np.int64)
    dst = np.asarray(edge_index[1], np.int64)
    core_of = dst // NSH

    z1row = src + src // NSH
    z2row = (src // NSH) * NR2 + (src % NSH)
    q1 = z1row // WIN1
    q2 = z2row // WIN2

    lay1 = _Layer(NT, NSH)
    lay2 = _Layer(NT, NSH)

    per1, per2 = [], []
    for c in range(NCORES):
        m = core_of == c
        d_loc = dst[m] - c * NSH
        t_of = d_loc // P
        per1.append(dict(t_of=t_of, d_loc=d_loc, q=q1[m], z=z1row[m] - q1[m] * WIN1))
        per2.append(dict(t_of=t_of, d_loc=d_loc, q=q2[m], z=z2row[m] - q2[m] * WIN2))
        for ed, NQn in ((per1[-1], NQ), (per2[-1], NQ)):
            cnt = np.zeros((NT, NQn), np.int64)
            np.add.at(cnt, (ed["t_of"], ed["q"]), 1)
            ed["cnt"] = cnt

    lay1.build([e["cnt"] for e in per1])
    lay2.build([e["cnt"] for e in per2])
    gidx1, dsl1, slh1 = _build_layer_tables(lay1, per1, NSH)
    gidx2, dsl2, slh2 = _build_layer_tables(lay2, per2, NPAD)

    xT = np.ascontiguousarray(x.T)
    in_maps = []
    for c in range(NCORES):
        xsh = np.zeros((F, NPAD), np.float32)
        xsh[:, :NSH] = xT[:, c * NSH:(c + 1) * NSH]
        in_maps.append({
            "xT": _bf16(xsh), "w1p": _bf16(W1p), "w2p": _bf16(W2p),
            "b1": b1rep, "b2": b2rep, "iota": iota, "iota2": iota2,
            "identb": _bf16(identb),
            "gidx1": gidx1[c], "dslot1": dsl1[c], "dslh1": slh1[c],
            "gidx2": gidx2[c], "dslot2": dsl2[c], "dslh2": slh2[c],
        })

    meta = dict(
        N=N, F=F, HID=HID, H1=H1, O1=O1, C=C, NSH=NSH, NT=NT, NPAD=NPAD,
        NR1=NR1, NR2=NR2, ZROW=ZROW, WIN1=WIN1, WIN2=WIN2,
        PAY1=PAY1, PAY2=PAY2, lay1=lay1, lay2=lay2,
    )
    return meta, in_maps


def _build(meta):
    F, HID, H1, O1, C = meta["F"], meta["HID"], meta["H1"], meta["O1"], meta["C"]
    NSH, NT, NPAD = meta["NSH"], meta["NT"], meta["NPAD"]
    NR1, NR2, ZROW = meta["NR1"], meta["NR2"], meta["ZROW"]
    WIN1, WIN2 = meta["WIN1"], meta["WIN2"]
    PAY1, PAY2 = meta["PAY1"], meta["PAY2"]
    lay1, lay2 = meta["lay1"], meta["lay2"]
    KCH = F // P
    ZC1 = HID + 2 * H1          # 80 used cols in z1 rows
    ZC2 = C + 2                 # 18 used cols in z2 rows

    nc = bacc.Bacc(
        "TRN2", target_bir_lowering=False, debug=False,
        enable_asserts=False, num_devices=NCORES,
    )

    xT = nc.dram_tensor("xT", [F, NPAD], BF16, kind="ExternalInput")
    w1p = nc.dram_tensor("w1p", [F, ZC1], BF16, kind="ExternalInput")
    w2p = nc.dram_tensor("w2p", [HID, ZC2], BF16, kind="ExternalInput")
    b1 = nc.dram_tensor("b1", [P, HID], F32, kind="ExternalInput")
    b2 = nc.dram_tensor("b2", [P, C], F32, kind="ExternalInput")
    iot = nc.dram_tensor("iota", [P, P], F32, kind="ExternalInput")
    iot2 = nc.dram_tensor("iota2", [P, P], F32, kind="ExternalInput")
    idnb = nc.dram_tensor("identb", [P, P], BF16, kind="ExternalInput")
    gidx1 = nc.dram_tensor("gidx1", [P, lay1.G], I16, kind="ExternalInput")
    dslot1 = nc.dram_tensor("dslot1", [P, lay1.S], F32, kind="ExternalInput")
    dslh1 = nc.dram_tensor("dslh1", [P, 4 * lay1.S], F32, kind="ExternalInput")
    gidx2 = nc.dram_tensor("gidx2", [P, lay2.G], I16, kind="ExternalInput")
    dslot2 = nc.dram_tensor("dslot2", [P, lay2.S], F32, kind="ExternalInput")
    dslh2 = nc.dram_tensor("dslh2", [P, 4 * lay2.S], F32, kind="ExternalInput")
    out_sm = nc.dram_tensor("out_sm", [NPAD, C], F32, kind="ExternalOutput")
    out_emb = nc.dram_tensor("out_emb", [NPAD, C], F32, kind="ExternalOutput")

    z1sh = nc.dram_tensor("z1sh", [NR1, ZROW], BF16)
    z1full = nc.dram_tensor("z1full", [NCORES * NR1, ZROW], BF16,
                            addr_space="Shared")
    z2sh = nc.dram_tensor("z2sh", [NR2, ZROW], BF16)
    z2full = nc.dram_tensor("z2full", [NCORES * NR2, ZROW], BF16,
                            addr_space="Shared")
    rg = [list(range(NCORES))]

    with tile.TileContext(nc, num_cores=NCORES) as tc, ExitStack() as ctx:
        cpool = ctx.enter_context(tc.tile_pool(name="const", bufs=1))
        w1sb = cpool.tile([P, KCH, ZC1], BF16)
        nc.sync.dma_start(w1sb[:], w1p[:].rearrange("(k p) o -> p k o", p=P))
        w2sb = cpool.tile([HID, ZC2], BF16)
        nc.sync.dma_start(w2sb[:], w2p[:])
        b1sb = cpool.tile([P, HID], F32)
        nc.sync.dma_start(b1sb[:], b1[:])
        b2sb = cpool.tile([P, C], F32)
        nc.sync.dma_start(b2sb[:], b2[:])
        iosb = cpool.tile([P, P], F32)
        nc.sync.dma_start(iosb[:], iot[:])
        io2sb = cpool.tile([P, P], F32)
        nc.sync.dma_start(io2sb[:], iot2[:])
        idsb = cpool.tile([P, P], BF16)
        nc.sync.dma_start(idsb[:], idnb[:])

        # pad rows (window-local pad index points here; asrc = PADV)
        pr1 = cpool.tile([1, ZROW], BF16)
        nc.vector.memset(pr1[:], 0.0)
        nc.vector.memset(pr1[:, HID:HID + H1], PADV)
        nc.sync.dma_start(z1sh[NSH:NSH + 1, :], pr1[:])
        pr2 = cpool.tile([1, ZROW], BF16)
        nc.vector.memset(pr2[:], 0.0)
        nc.vector.memset(pr2[:, C:C + 1], PADV)
        nc.sync.dma_start(z2sh[NPAD:NPAD + 1, :], pr2[:])

        # ---------------- phase A: layer-1 node projection ----------------
        with (
            tc.tile_pool(name="nodeA", bufs=3) as pa,
            tc.tile_pool(name="psumA", bufs=2, space="PSUM") as ppa,
        ):
            for t in range(NT):
                xt = pa.tile([P, KCH, P], BF16, tag="xt")
                nc.sync.dma_start(
                    xt[:], xT[:, t * P:(t + 1) * P].rearrange("(k p) n -> p k n", p=P)
                )
                ps = ppa.tile([P, ZC1], F32, tag="h1ps")
                for k in range(KCH):
                    nc.tensor.matmul(
                        ps[:], lhsT=xt[:, k, :], rhs=w1sb[:, k, :],
                        start=(k == 0), stop=(k == KCH - 1),
                    )
                zt = pa.tile([P, ZROW], BF16, tag="zt")
                nc.scalar.copy(zt[:, 0:ZC1], ps[:])
                nc.vector.memset(zt[:, ZC1:], 0.0)
                rows = min(NSH - t * P, P)
                nc.sync.dma_start(z1sh[t * P:t * P + rows, :], zt[:rows, :])

        PH = int(os.environ.get("GAT_PHASES", "5"))
        if PH >= 2:
            nc.gpsimd.collective_compute(
                "AllGather", ALU.bypass, replica_groups=rg,
                ins=[z1sh[:]], outs=[z1full[:]]
            )

        # ---------------- edge phase (shared for both layers) ----------------
        def edge_phase2(lay, zfull, zloc, WIN, PAY, gidx, dslot, dslh,
                        NH, OC, layer):
            VAL = NH + NH * OC
            ADW = max(NH, 2)
            AD0 = PAY                # adst col in z rows
            goff, soff = 0, 0
            with ExitStack() as ectx:
                pg = ectx.enter_context(tc.tile_pool(name=f"eg{layer}", bufs=2))
                ps_ = ectx.enter_context(tc.tile_pool(name=f"es{layer}", bufs=2))
                pb = ectx.enter_context(tc.tile_pool(name=f"eb{layer}", bufs=2))
                pacc = ectx.enter_context(
                    tc.tile_pool(name=f"pacc{layer}", bufs=4, space="PSUM"))
                pad_ = ectx.enter_context(
                    tc.tile_pool(name=f"padst{layer}", bufs=2, space="PSUM"))
                if layer == 1:
                    ph2 = ectx.enter_context(
                        tc.tile_pool(name="ph2", bufs=1, space="PSUM"))
                    pg2 = ectx.enter_context(
                        tc.tile_pool(name="pg2", bufs=1, space="PSUM"))
                for sg in lay.sgs:
                    nch, ngath = sg["nch"], sg["ngath"]
                    tiles, rows_l = sg["tiles"], sg["rows"]
                    ctile = sg["chunk_tile"]
                    # --- tables ---
                    sl = ps_.tile([P, nch], F32, tag="sl")
                    nc.sync.dma_start(sl[:], dslot[:, soff:soff + nch])
                    slh = ps_.tile([P, nch, NQ], F32, tag="slh")
                    nc.sync.dma_start(
                        slh[:].rearrange("p n h -> p (n h)"),
                        dslh[:, 4 * soff:4 * (soff + nch)])
                    soff += nch
                    # --- local dst data: a_dst per tile ---
                    adsg = ps_.tile([P, TG, ADW], BF16, tag="adsg")
                    nc.vector.memset(adsg[:], 0.0)
                    for i, t in enumerate(tiles):
                        r = rows_l[i]
                        nc.sync.dma_start(
                            adsg[0:r, i, 0:NH],
                            zloc[t * P:t * P + r, AD0:AD0 + NH])
                    # --- gathers (Q7) per window ---
                    gg = pg.tile([P, nch, PAY], BF16, tag="gg")
                    off = 0
                    for q in range(NQ):
                        cq = sg["qruns"].get(q, 0)
                        if cq == 0:
                            continue
                        n = cq * P
                        ixw = ps_.tile([P, n // 16], I16, tag="ixw")
                        nc.sync.dma_start(ixw[:], gidx[:, goff:goff + n // 16])
                        goff += n // 16
                        dma_gather_rows(
                            nc.gpsimd,
                            out_ap=gg[:, off:off + cq, :],
                            in_ap=zfull[q * WIN:(q + 1) * WIN, 0:PAY],
                            idxs_ap=ixw[:],
                            num_idxs=n,
                            elem_size=PAY,
                            elem_step=ZROW,
                            queue_num=0,
                        )
                        off += cq
                    # --- self-loop chunks: direct DMA from local shard ---
                    for i, t in enumerate(tiles):
                        r = rows_l[i]
                        if r < P:
                            nc.vector.memset(gg[:, ngath + i, :], 0.0)
                        nc.sync.dma_start(
                            gg[0:r, ngath + i, :],
                            zloc[t * P:t * P + r, 0:PAY])
                    EP = int(os.environ.get("GAT_EDGE", "5"))
                    if EP < 2:
                        continue
                    # --- attention + scatter, batched SB chunks ---
                    started = set()
                    remaining = {}
                    for tl in ctile:
                        remaining[tl] = remaining.get(tl, 0) + 1
                    accs = {}
                    val = pg.tile([P, nch, VAL], BF16, tag="val")
                    for c0 in range(0, nch, SB):
                        cb = min(SB, nch - c0)
                        st = ps_.tile([P, SB, P], BF16, tag="st")
                        nc.vector.tensor_tensor(
                            out=st[:, :cb, :],
                            in0=iosb[:][:, None, :].to_broadcast([P, cb, P]),
                            in1=sl[:, c0:c0 + cb][:, :, None].to_broadcast(
                                [P, cb, P]),
                            op=ALU.is_equal,
                        )
                        sI = ps_.tile([P, SB, P], BF16, tag="sI")
                        nc.vector.tensor_tensor(
                            out=sI[:, :cb, :].rearrange(
                                "p n (h b) -> p n h b", h=NQ),
                            in0=io2sb[:].rearrange("p (h b) -> p h b", h=NQ)[
                                :, None, :, :].to_broadcast([P, cb, NQ, 32]),
                            in1=slh[:, c0:c0 + cb, :][:, :, :, None].to_broadcast(
                                [P, cb, NQ, 32]),
                            op=ALU.is_equal,
                        )
                        stT = ps_.tile([P, SB, P], BF16, tag="stT")
                        nc.vector.transpose(
                            stT[:, :cb, :].rearrange("p n e -> p (n e)"),
                            sI[:, :cb, :].rearrange("p n e -> p (n e)"))
                        # a_dst expand: psum[:, j, :] = stT_j^T-contract @ adsg
                        adps = pad_.tile([P, SB, ADW], F32, tag="adps")
                        for j in range(cb):
                            nc.tensor.matmul(
                                adps[:, j, :],
                                lhsT=stT[:, j, :],
                                rhs=adsg[:, ctile[c0 + j], :],
                                start=True, stop=True,
                            )
                        adb = ps_.tile([P, SB, NH], BF16, tag="adb")
                        nc.scalar.copy(adb[:, :cb, :], adps[:, :cb, 0:NH])
                        if EP < 3:
                            continue
                        # z = asrc + adst ; alpha = 0.6 z + 0.4|z| ; exp
                        zt = ps_.tile([P, SB, NH], F32, tag="zt")
                        nc.vector.tensor_tensor(
                            out=zt[:, :cb, :],
                            in0=gg[:, c0:c0 + cb, NH * OC:NH * OC + NH],
                            in1=adb[:, :cb, :],
                            op=ALU.add,
                        )
                        ab = ps_.tile([P, SB, NH], F32, tag="ab")
                        nc.scalar.activation(ab[:, :cb, :], zt[:, :cb, :],
                                             ACTF.Abs, scale=0.4)
                        al = ps_.tile([P, SB, NH], F32, tag="al")
                        nc.vector.scalar_tensor_tensor(
                            out=al[:, :cb, :], in0=zt[:, :cb, :], scalar=0.6,
                            in1=ab[:, :cb, :], op0=ALU.mult, op1=ALU.add,
                        )
                        nc.scalar.activation(
                            val[:, c0:c0 + cb, 0:NH], al[:, :cb, :], ACTF.Exp)
                        nc.vector.tensor_tensor(
                            out=val[:, c0:c0 + cb, NH:].rearrange(
                                "p n (h c) -> p n h c", h=NH),
                            in0=gg[:, c0:c0 + cb, 0:NH * OC].rearrange(
                                "p n (h c) -> p n h c", h=NH),
                            in1=val[:, c0:c0 + cb, 0:NH][:, :, :, None
                                                         ].to_broadcast(
                                [P, cb, NH, OC]),
                            op=ALU.mult,
                        )
                        if EP < 4:
                            continue
                        for j in range(cb):
                            tl = ctile[c0 + j]
                            if tl not in accs:
                                accs[tl] = pacc.tile(
                                    [P, VAL], F32, tag="acc",
                                    name=f"acc_l{layer}_t{tiles[tl]}")
                            remaining[tl] -= 1
                            nc.tensor.matmul(
                                accs[tl][:],
                                lhsT=st[:, j, :],
                                rhs=val[:, c0 + j, :],
                                start=(tl not in started),
                                stop=(remaining[tl] == 0),
                            )
                            started.add(tl)
                    if EP < 5:
                        continue
                    # --- batched postprocess for the sg ---
                    ntl = len(tiles)
                    zb = pb.tile([P, TG, VAL], F32, tag="zb")
                    for tl in range(ntl):
                        nc.scalar.copy(zb[:, tl, :], accs[tl][:])
                    s2 = pb.tile([P, TG, NH], F32, tag="s2")
                    nc.vector.tensor_scalar_add(
                        s2[:, :ntl, :], zb[:, :ntl, 0:NH], 1e-16)
                    rc = pb.tile([P, TG, NH], F32, tag="rc")
                    nc.vector.reciprocal(rc[:, :ntl, :], s2[:, :ntl, :])
                    o1 = pb.tile([P, TG, NH, OC], F32, tag="o1")
                    nc.vector.tensor_tensor(
                        out=o1[:, :ntl],
                        in0=zb[:, :ntl, NH:].rearrange(
                            "p t (h c) -> p t h c", h=NH),
                        in1=rc[:, :ntl, :, None].to_broadcast([P, ntl, NH, OC]),
                        op=ALU.mult,
                    )
                    if layer == 1:
                        o2 = pb.tile([P, TG, HID], F32, tag="o2")
                        nc.vector.tensor_tensor(
                            out=o2[:, :ntl],
                            in0=o1[:, :ntl].rearrange("p t h c -> p t (h c)"),
                            in1=b1sb[:][:, None, :].to_broadcast([P, ntl, HID]),
                            op=ALU.add,
                        )
                        mn = pb.tile([P, TG, HID], F32, tag="mn")
                        nc.vector.tensor_scalar_min(
                            mn[:, :ntl], o2[:, :ntl], 0.0)
                        exm = pb.tile([P, TG, HID], F32, tag="exm")
                        nc.scalar.activation(exm[:, :ntl], mn[:, :ntl], ACTF.Exp)
                        h2a = pb.tile([P, TG, HID], F32, tag="h2a")
                        nc.vector.scalar_tensor_tensor(
                            out=h2a[:, :ntl], in0=o2[:, :ntl], scalar=0.0,
                            in1=exm[:, :ntl], op0=ALU.max, op1=ALU.add,
                        )
                        h2t = pb.tile([P, TG, HID], BF16, tag="h2t")
                        nc.vector.tensor_scalar_add(
                            h2t[:, :ntl], h2a[:, :ntl], -1.0)
                        for tl, t in enumerate(tiles):
                            tp = ph2.tile([HID, P], BF16, tag="h2T")
                            nc.tensor.transpose(tp[:], h2t[:, tl, :], idsb[:])
                            h2T = pb.tile([HID, P], BF16, tag="h2Ts")
                            nc.scalar.copy(h2T[:], tp[:])
                            g2p = pg2.tile([P, ZC2], F32, tag="g2ps")
                            nc.tensor.matmul(
                                g2p[:], lhsT=h2T[:], rhs=w2sb[:],
                                start=True, stop=True,
                            )
                            z2t = pb.tile([P, ZROW], BF16, tag="z2t")
                            nc.scalar.copy(z2t[:, 0:ZC2], g2p[:])
                            nc.vector.memset(z2t[:, ZC2:], 0.0)
                            nc.sync.dma_start(
                                z2sh[t * P:(t + 1) * P, :], z2t[:])
                    else:
                        emb = pb.tile([P, TG, C], F32, tag="emb")
                        nc.vector.tensor_tensor(
                            out=emb[:, :ntl],
                            in0=o1[:, :ntl].rearrange("p t h c -> p t (h c)"),
                            in1=b2sb[:][:, None, :].to_broadcast([P, ntl, C]),
                            op=ALU.add,
                        )
                        mxn = pb.tile([P, TG, 1], F32, tag="mxn")
                        nc.vector.tensor_reduce(
                            mxn[:, :ntl], emb[:, :ntl], axis=AX.X, op=ALU.max,
                            negate=True)
                        esh = pb.tile([P, TG, C], F32, tag="esh")
                        nc.vector.tensor_tensor(
                            out=esh[:, :ntl], in0=emb[:, :ntl],
                            in1=mxn[:, :ntl, :].to_broadcast([P, ntl, C]),
                            op=ALU.add,
                        )
                        es = pb.tile([P, TG, C], F32, tag="es")
                        nc.scalar.activation(es[:, :ntl], esh[:, :ntl], ACTF.Exp)
                        ssum = pb.tile([P, TG, 1], F32, tag="ssum")
                        nc.vector.tensor_reduce(
                            ssum[:, :ntl], es[:, :ntl], axis=AX.X, op=ALU.add)
                        rr = pb.tile([P, TG, 1], F32, tag="rr")
                        nc.vector.reciprocal(rr[:, :ntl], ssum[:, :ntl])
                        sm = pb.tile([P, TG, C], F32, tag="sm")
                        nc.vector.tensor_tensor(
                            out=sm[:, :ntl], in0=es[:, :ntl],
                            in1=rr[:, :ntl, :].to_broadcast([P, ntl, C]),
                            op=ALU.mult,
                        )
                        for tl, t in enumerate(tiles):
                            nc.sync.dma_start(
                                out_emb[t * P:(t + 1) * P, :], emb[:, tl, :])
                            nc.sync.dma_start(
                                out_sm[t * P:(t + 1) * P, :], sm[:, tl, :])
            assert goff == lay.G and soff == lay.S, (goff, lay.G, soff, lay.S)

        if PH >= 3:
            edge_phase2(lay1, z1full, z1sh, WIN1, PAY1, gidx1, dslot1, dslh1,
                        H1, O1, layer=1)

        if PH >= 4:
            nc.gpsimd.collective_compute(
                "AllGather", ALU.bypass, replica_groups=rg,
                ins=[z2sh[:]], outs=[z2full[:]]
            )

        if PH >= 5:
            edge_phase2(lay2, z2full, z2sh, WIN2, PAY2, gidx2, dslot2, dslh2,
                        1, C, layer=2)

    nc.compile()
    return nc


def _postprocess(meta, results):
    NSH = meta["NSH"]
    outs, embs = [], []
    for c in range(NCORES):
        outs.append(results[c]["out_sm"][:NSH])
        embs.append(results[c]["out_emb"][:NSH])
    return np.concatenate(outs, 0), np.concatenate(embs, 0)


def kernel(**inputs):
    meta, in_maps = _prep(**inputs)
    nc = _build(meta)
    from concourse.bass_utils import run_bass_kernel_spmd

    trace = bool(int(os.environ.get("GAT_TRACE", "0")))
    res = run_bass_kernel_spmd(nc, in_maps, list(range(NCORES)), trace=trace)
    if trace and res.exec_time_ns is not None:
        print(f"HW exec time: {res.exec_time_ns} ns")
        kernel.last_exec_time_ns = res.exec_time_ns
    return _postprocess(meta, res.results)
